# revision 4
# baseline (speedup 1.0000x reference)
"""MultiHeadAttention Trainium2 kernel.

Full inputs -> full output. Sharding: 8 cores = (batch b in 0..3) x (head
group hg in 0..1). Each core projects Q/K/V for its 8 heads (columns
hg*512..hg*512+512 of wq/wk/wv) over batch b's full 2048 rows, runs
attention for those heads, and applies its half of the output projection
(rows hg*512.. of wo). Host sums the two partial outputs per batch.

All matmuls in float32r (1 cycle/row for N>=256, ~2^-12 rounding).

  phase A: K^T [512,2048] and V [2048,512(+ones col)] stay resident in
           SBUF; Q^T spills to DRAM. Biases bq/bk are added by the Act
           engine during the PSUM->SBUF copies (no rank-1 matmuls).
  phase B: software-pipelined (head, query-block) iterations: logits^T
           [sk,sq] = K_h^T.T @ Q_h^T (64-dim contraction at the head's
           base partition), P = exp(0.125*logits + mask*(-1e9)) on the
           Act engine, interleaved on the PE with the previous
           iteration's ctx matmuls (V augmented with a ones column so
           the softmax denominator falls out); raw ctx^T rows -> DRAM.
  phase C: reciprocal of the 32 denominator rows, PE broadcast of the
           recips, normalize raw ctx^T, out = ctx @ wo_half + bias via a
           DVE add (bias = bv_half@wo_half (+bo on hg==0), host-folded).
"""

import numpy as np

import concourse.bass as bass
import concourse.mybir as mybir
import concourse.tile as tile
from concourse import bacc
from concourse.bass_utils import run_bass_kernel_spmd

f32 = mybir.dt.float32
f32r = mybir.dt.float32r

B, S, D, H, DH = 4, 2048, 1024, 16, 64
HD2 = D // 2         # 512 columns per head group
N_CORES = 8
Exp = mybir.ActivationFunctionType.Exp
Ident = mybir.ActivationFunctionType.Identity

KC = D // 128        # 8 contraction chunks over the model dim
OC = HD2 // 128      # 4 output chunks (local head pairs)
SKC = S // 128       # 16 key chunks
XB = 8               # x streamed in 8 blocks of 256 seq positions
NT = 8 * 4           # 32 pipelined iterations (local head, query block)


def _iter_map():
    """Emission order of phase-B iterations -> (local head, query block)."""
    seq = []
    for ko in range(OC):
        for sq2 in range(2):
            for h2 in range(2):
                for sqs in range(2):
                    seq.append((ko * 2 + h2, sq2 * 2 + sqs))
    return seq


def _build():
    nc = bacc.Bacc(None, target_bir_lowering=False)

    # pre-chunked host layouts (see kernel() below)
    xq = nc.dram_tensor("xq", [XB, 128, KC, 256], f32r, kind="ExternalInput")
    xk = nc.dram_tensor("xk", [XB, 128, KC, 256], f32r, kind="ExternalInput")
    xv = nc.dram_tensor("xv", [XB, 128, KC, 256], f32r, kind="ExternalInput")
    wq = nc.dram_tensor("wq", [128, KC, HD2], f32r, kind="ExternalInput")
    wk = nc.dram_tensor("wk", [128, KC, HD2], f32r, kind="ExternalInput")
    wv = nc.dram_tensor("wv", [128, KC, HD2], f32r, kind="ExternalInput")
    wo = nc.dram_tensor("wo", [128, OC, D], f32r, kind="ExternalInput")
    bq = nc.dram_tensor("bq", [128, OC], f32, kind="ExternalInput")
    bk = nc.dram_tensor("bk", [128, OC], f32, kind="ExternalInput")
    bo = nc.dram_tensor("bo", [128, D], f32, kind="ExternalInput")
    mb = nc.dram_tensor("mb", [128, SKC], f32, kind="ExternalInput")   # mask*-1e9
    sel = nc.dram_tensor("sel", [4, NT, OC, 128], f32r, kind="ExternalInput")
    out = nc.dram_tensor("out", [S, D], f32, kind="ExternalOutput")

    with tile.TileContext(nc) as tc:
        _emit(nc, tc, xq, xk, xv, wq, wk, wv, wo, bq, bk, bo, mb, sel, out)
    nc.finalize()
    return nc


def _emit(nc, tc, xq, xk, xv, wq, wk, wv, wo, bq, bk, bo, mb, sel, out):
    from contextlib import ExitStack

    with ExitStack() as ctx:
        consts = ctx.enter_context(tc.tile_pool(name="consts", bufs=1))
        kvres = ctx.enter_context(tc.tile_pool(name="kvres", bufs=1))
        wpool = ctx.enter_context(tc.tile_pool(name="wpool", bufs=2))
        xtp = ctx.enter_context(tc.tile_pool(name="xtp", bufs=2))
        qts = ctx.enter_context(tc.tile_pool(name="qts", bufs=2))
        ptp = ctx.enter_context(tc.tile_pool(name="ptp", bufs=9))
        stg = ctx.enter_context(tc.tile_pool(name="stg", bufs=2))
        stg2 = ctx.enter_context(tc.tile_pool(name="stg2", bufs=2))
        rbp = ctx.enter_context(tc.tile_pool(name="rbp", bufs=2))
        selp = ctx.enter_context(tc.tile_pool(name="selp", bufs=2))
        ctp = ctx.enter_context(tc.tile_pool(name="ctp", bufs=2))
        psA = ctx.enter_context(tc.tile_pool(name="psA", bufs=3, space="PSUM"))
        psC = ctx.enter_context(tc.tile_pool(name="psC", bufs=2, space="PSUM"))
        dram = ctx.enter_context(tc.tile_pool(name="dram", bufs=1, space="DRAM"))

        qtd = dram.tile([HD2, S], f32r)     # Q^T spill
        crd = dram.tile([HD2, S], f32r)     # raw (unnormalized) ctx^T
        dnd = dram.tile([NT, 512], f32r)    # denominator rows

        mb_sb = consts.tile([128, SKC], f32)
        nc.sync.dma_start(mb_sb, mb[:])
        bq_sb = consts.tile([128, OC], f32)
        nc.sync.dma_start(bq_sb, bq[:])
        bk_sb = consts.tile([128, OC], f32)
        nc.sync.dma_start(bk_sb, bk[:])

        kT = kvres.tile([128, OC, S], f32r)          # K^T resident
        va = kvres.tile([128, SKC, 8, DH + 1], f32r)  # V + ones col resident
        nc.vector.memset(va[:, :, :, DH].bitcast(f32), 1.0)

        # ================= phase A =================
        wq_sb = wpool.tile([128, KC, HD2], f32r, tag="w", name="wq_sb")
        nc.sync.dma_start(wq_sb, wq[:])
        for blk in range(XB):
            xt = xtp.tile([128, KC, 256], f32r, tag="xT", name="xt")
            nc.sync.dma_start(xt, xq[blk])
            for dc2 in range(2):
                ps = psA.tile([128, 2, 512], f32, tag="psA", name="ps")
                stq = stg.tile([128, 2, 512], f32r, tag="stg", name="stq")
                for half in range(2):
                    dc = dc2 * 2 + half
                    for kc in range(KC):
                        nc.tensor.matmul(ps[:, half, 0:256],
                                         lhsT=wq_sb[:, kc, dc * 128:(dc + 1) * 128],
                                         rhs=xt[:, kc, :],
                                         start=(kc == 0), stop=(kc == KC - 1))
                    nc.scalar.activation(stq[:, half, 0:256], ps[:, half, 0:256],
                                         Ident, bias=bq_sb[:, dc:dc + 1])
                    nc.sync.dma_start(
                        qtd[dc * 128:(dc + 1) * 128, blk * 256:(blk + 1) * 256],
                        stq[:, half, 0:256])

        wk_sb = wpool.tile([128, KC, HD2], f32r, tag="w", name="wk_sb")
        nc.sync.dma_start(wk_sb, wk[:])
        for blk in range(XB):
            xt = xtp.tile([128, KC, 256], f32r, tag="xT", name="xt")
            nc.sync.dma_start(xt, xk[blk])
            for dc2 in range(2):
                ps = psA.tile([128, 2, 512], f32, tag="psA", name="ps")
                for half in range(2):
                    dc = dc2 * 2 + half
                    for kc in range(KC):
                        nc.tensor.matmul(ps[:, half, 0:256],
                                         lhsT=wk_sb[:, kc, dc * 128:(dc + 1) * 128],
                                         rhs=xt[:, kc, :],
                                         start=(kc == 0), stop=(kc == KC - 1))
                    nc.scalar.activation(
                        kT[:, dc, blk * 256:(blk + 1) * 256],
                        ps[:, half, 0:256], Ident, bias=bk_sb[:, dc:dc + 1])

        wv_sb = wpool.tile([128, KC, HD2], f32r, tag="w", name="wv_sb")
        nc.sync.dma_start(wv_sb, wv[:])
        for blk in range(XB):
            xt = xtp.tile([128, KC, 256], f32r, tag="xT", name="xvt")
            nc.sync.dma_start(xt, xv[blk])
            for sub in range(2):
                sc = blk * 2 + sub
                ps = psC.tile([128, 512], f32, tag="psC", name="psv")
                for kc in range(KC):
                    nc.tensor.matmul(ps,
                                     lhsT=xt[:, kc, sub * 128:(sub + 1) * 128],
                                     rhs=wv_sb[:, kc, :],
                                     start=(kc == 0), stop=(kc == KC - 1))
                with nc.allow_low_precision(reason="V rounded to f32r"):
                    nc.vector.tensor_copy(va[:, sc, :, 0:DH],
                                          ps.rearrange("p (h d) -> p h d", h=8))

        # wo shares the weight ring (viewed [128, OC, 1024]); bo broadcast
        wo_sb_raw = wpool.tile([128, KC, HD2], f32r, tag="w", name="wo_sb")
        nc.sync.dma_start(wo_sb_raw.rearrange("p a b -> p (a b)"),
                          wo[:].rearrange("p a b -> p (a b)"))
        wo_sb = wo_sb_raw.rearrange("p (a c) b -> p a (c b)", c=2)
        bo_sb = consts.tile([128, D], f32)
        nc.sync.dma_start(bo_sb, bo[:])

        # ================= phase B: software-pipelined attention =========
        state = {}

        def emit_logits_pair(t, sc2):
            st_ = state[t]
            b0 = st_["b0"]
            psl = psA.tile([128, 2, 512], f32, tag="psA", name="psl")
            for half in range(2):
                sc = sc2 * 2 + half
                nc.tensor.matmul(
                    psl[:, half, :],
                    lhsT=kT[b0:b0 + 64, st_["ko"], sc * 128:(sc + 1) * 128],
                    rhs=st_["qt"][b0:b0 + 64,
                                  st_["sqs"] * 512:(st_["sqs"] + 1) * 512],
                    start=True, stop=True)
            pt = ptp.tile([128, 2, 512], f32r, tag="pt", name="pt")
            nc.scalar.activation(
                pt.rearrange("p a b -> p (a b)"),
                psl.rearrange("p a b -> p (a b)"), Exp,
                bias=mb_sb[:, sc2 * 2:sc2 * 2 + 1], scale=0.125)
            st_["pt"].append(pt)

        def emit_ctx_chunk(t, sc):
            st_ = state[t]
            if sc == 0:
                st_["psc"] = psC.tile([128, 512], f32, tag="psC", name="psc")
            nc.tensor.matmul(st_["psc"][0:DH + 1, :],
                             lhsT=va[:, sc, st_["h"], :],
                             rhs=st_["pt"][sc // 2][:, sc % 2, :],
                             start=(sc == 0), stop=(sc == SKC - 1))

        def emit_store(t):
            st_ = state[t]
            h, sqb = st_["h"], st_["sqb"]
            cu = stg2.tile([65, 512], f32r, tag="cu", name="cu")
            with nc.allow_low_precision(reason="raw ctx rounded to f32r"):
                nc.vector.tensor_copy(cu, st_["psc"][0:DH + 1, :])
            nc.sync.dma_start(crd[h * 64:(h + 1) * 64, sqb * 512:(sqb + 1) * 512],
                              cu[0:DH, :])
            nc.sync.dma_start(dnd[t:t + 1, :], cu[DH:DH + 1, :])
            del state[t]

        imap = _iter_map()
        cur_qt = None
        for t in range(NT):
            h, sqb = imap[t]
            ko, h2 = divmod(h, 2)
            if h2 == 0 and sqb % 2 == 0:
                cur_qt = qts.tile([128, 1024], f32r, tag="qt", name="qt")
                nc.sync.dma_start(
                    cur_qt,
                    qtd[ko * 128:(ko + 1) * 128,
                        (sqb // 2) * 1024:(sqb // 2 + 1) * 1024])
            state[t] = {"h": h, "sqb": sqb, "ko": ko, "b0": h2 * 64,
                        "sqs": sqb % 2, "qt": cur_qt, "pt": []}

            for sc2 in range(SKC // 2):
                emit_logits_pair(t, sc2)
                if t >= 1:
                    emit_ctx_chunk(t - 1, sc2 * 2)
                    emit_ctx_chunk(t - 1, sc2 * 2 + 1)
            if t >= 1:
                emit_store(t - 1)

        for sc in range(SKC):
            emit_ctx_chunk(NT - 1, sc)
        emit_store(NT - 1)

        # ================= phase C: normalize + output projection =========
        den_sb = consts.tile([NT, 512], f32r)
        nc.sync.dma_start(den_sb, dnd[:])
        recf = consts.tile([NT, 512], f32)
        nc.vector.reciprocal(recf, den_sb)
        rec = consts.tile([NT, 512], f32r)
        with nc.allow_low_precision(reason="softmax recip rounded to f32r"):
            nc.vector.tensor_copy(rec, recf)

        for sqb in range(4):
            sel_t = selp.tile([NT, OC, 128], f32r, tag="sel", name="sel_t")
            nc.sync.dma_start(sel_t, sel[sqb])
            rbt = rbp.tile([128, OC, 512], f32r, tag="rb", name="rbt")
            for ko in range(OC):
                pb = psC.tile([128, 512], f32, tag="psC", name="pb")
                nc.tensor.matmul(pb, lhsT=sel_t[:, ko, :], rhs=rec[:],
                                 start=True, stop=True)
                with nc.allow_low_precision(reason="recip bcast in f32r"):
                    nc.vector.tensor_copy(rbt[:, ko, :], pb)
            for st4 in range(4):
                st8 = sqb * 4 + st4
                cT = ctp.tile([128, OC, 128], f32r, tag="cT", name="cT")
                nc.sync.dma_start(cT, crd[:, st8 * 128:(st8 + 1) * 128]
                                  .rearrange("(ko p) q -> p ko q", p=128))
                with nc.allow_low_precision(reason="normalized ctx in f32r"):
                    nc.vector.tensor_mul(out=cT, in0=cT,
                                         in1=rbt[:, :, st4 * 128:(st4 + 1) * 128])
                ps = psA.tile([128, 2, 512], f32, tag="psA", name="pso")
                for half in range(2):
                    for ko in range(OC):
                        nc.tensor.matmul(
                            ps[:, half, :], lhsT=cT[:, ko, :],
                            rhs=wo_sb[:, ko, half * 512:(half + 1) * 512],
                            start=(ko == 0), stop=(ko == OC - 1))
                st_t = stg.tile([128, 2, 512], f32r, tag="stg", name="ost")
                with nc.allow_low_precision(reason="f32r storage is fp32 bits"):
                    nc.vector.tensor_add(
                        out=st_t.rearrange("p a b -> p (a b)"),
                        in0=ps.rearrange("p a b -> p (a b)"), in1=bo_sb)
                nc.sync.dma_start(out[st8 * 128:(st8 + 1) * 128, :],
                                  st_t.rearrange("p a b -> p (a b)").bitcast(f32))


_NC_CACHE = None


def _selector():
    # sel[sqb, t, ko, p] = 1 iff phase-B iteration t produced denominators
    # for (head ko*2 + p//64, query block sqb)
    imap = _iter_map()
    s = np.zeros((4, NT, OC, 128), np.float32)
    for t, (h, sqb) in enumerate(imap):
        ko, h2 = divmod(h, 2)
        s[sqb, t, ko, h2 * 64:(h2 + 1) * 64] = 1.0
    return s


def kernel(query, key, value, mask, wq, bq, wk, bk, wv, bv, wo, bo):
    global _NC_CACHE
    if _NC_CACHE is None:
        _NC_CACHE = _build()
    nc = _NC_CACHE

    query = np.asarray(query, dtype=np.float32)
    key = np.asarray(key, dtype=np.float32)
    value = np.asarray(value, dtype=np.float32)
    mask = np.asarray(mask, dtype=np.float32)
    wq_np = np.asarray(wq, np.float32)
    wk_np = np.asarray(wk, np.float32)
    wv_np = np.asarray(wv, np.float32)
    wo_np = np.asarray(wo, np.float32)
    bq_np = np.asarray(bq, np.float32)
    bk_np = np.asarray(bk, np.float32)
    bv_np = np.asarray(bv, np.float64)
    bo_np = np.asarray(bo, np.float64)

    def chunk_x(xT):
        # [1024, 2048] -> [XB, 128, KC, 256] with d = kc*128 + p
        return np.ascontiguousarray(
            xT.reshape(KC, 128, XB, 256).transpose(2, 1, 0, 3))

    def chunk_w(w_half):
        # [1024, 512] -> [128, KC, 512]
        return np.ascontiguousarray(
            w_half.reshape(KC, 128, HD2).transpose(1, 0, 2))

    xq_b, xk_b, xv_b = [], [], []
    for b in range(B):
        xq_b.append(chunk_x(np.ascontiguousarray(query[b].T)))
        xk_b.append(chunk_x(np.ascontiguousarray(key[b].T)))
        xv_b.append(chunk_x(np.ascontiguousarray(value[b].T)))

    sel_host = _selector()
    in_maps = []
    for core in range(N_CORES):
        b, hg = divmod(core, 2)
        sl = slice(hg * HD2, (hg + 1) * HD2)
        bias_out = bv_np[sl] @ wo_np[sl].astype(np.float64)
        if hg == 0:
            bias_out = bias_out + bo_np
        mbc = np.ascontiguousarray(
            (mask[b, 0, 0] * np.float32(-1e9)).reshape(SKC, 128).T)
        in_maps.append({
            "xq": xq_b[b], "xk": xk_b[b], "xv": xv_b[b],
            "wq": chunk_w(wq_np[:, sl]),
            "wk": chunk_w(wk_np[:, sl]),
            "wv": chunk_w(wv_np[:, sl]),
            "wo": np.ascontiguousarray(
                wo_np[sl].reshape(OC, 128, D).transpose(1, 0, 2)),
            "bq": np.ascontiguousarray(bq_np[sl].reshape(OC, 128).T),
            "bk": np.ascontiguousarray(bk_np[sl].reshape(OC, 128).T),
            "bo": np.ascontiguousarray(
                np.broadcast_to(bias_out.astype(np.float32), (128, D))),
            "mb": mbc, "sel": sel_host,
        })

    res = run_bass_kernel_spmd(nc, in_maps, core_ids=list(range(N_CORES)))
    full = np.empty((B, S, D), np.float32)
    for b in range(B):
        full[b] = res.results[2 * b]["out"]
        full[b] += res.results[2 * b + 1]["out"]
    return full


# revision 8
# speedup vs baseline: 1.0713x; 1.0713x over previous
"""MultiHeadAttention Trainium2 kernel.

Full inputs -> full output. Sharding: 8 cores = (batch b in 0..3) x (head
group hg in 0..1). Each core projects Q/K/V for its 8 heads (columns
hg*512..hg*512+512 of wq/wk/wv) over batch b's full 2048 rows, runs
attention for those heads, and applies its half of the output projection
(rows hg*512.. of wo). Host sums the two partial outputs per batch.

All matmuls in float32r (1 cycle/row for N>=256, ~2^-12 rounding).

  phase A: K^T [512,2048] and V [2048,512(+ones col)] stay resident in
           SBUF; Q^T spills to DRAM. Biases bq/bk are added by the Act
           engine during the PSUM->SBUF copies (no rank-1 matmuls).
  phase B: software-pipelined (head, query-block) iterations: logits^T
           [sk,sq] = K_h^T.T @ Q_h^T (64-dim contraction at the head's
           base partition), P = exp(0.125*logits + mask*(-1e9)) on the
           Act engine, interleaved on the PE with the previous
           iteration's ctx matmuls (V augmented with a ones column so
           the softmax denominator falls out); raw ctx^T rows -> DRAM.
  phase C: reciprocal of the 32 denominator rows, PE broadcast of the
           recips, normalize raw ctx^T, out = ctx @ wo_half + bias via a
           DVE add (bias = bv_half@wo_half (+bo on hg==0), host-folded).
"""

import numpy as np

import concourse.bass as bass
import concourse.mybir as mybir
import concourse.tile as tile
from concourse import bacc
from concourse.bass_utils import run_bass_kernel_spmd

f32 = mybir.dt.float32
f32r = mybir.dt.float32r

B, S, D, H, DH = 4, 2048, 1024, 16, 64
HD2 = D // 2         # 512 columns per head group
N_CORES = 8
Exp = mybir.ActivationFunctionType.Exp
Ident = mybir.ActivationFunctionType.Identity

KC = D // 128        # 8 contraction chunks over the model dim
OC = HD2 // 128      # 4 output chunks (local head pairs)
SKC = S // 128       # 16 key chunks
XB = 8               # x streamed in 8 blocks of 256 seq positions
NT = 8 * 4           # 32 pipelined iterations (local head, query block)


def _iter_map():
    """Emission order of phase-B pair-iterations -> (head pair ko, query block).

    Each pair-iteration covers BOTH heads of pair ko (row-tiled on the PE).
    Denominator row t = 2*i + h2 for pair-iteration index i.
    """
    seq = []
    for sq2 in range(2):
        for ko in range(OC):
            for sqs in range(2):
                seq.append((ko, sq2 * 2 + sqs))
    return seq


def _build():
    nc = bacc.Bacc(None, target_bir_lowering=False)

    # pre-chunked host layouts (see kernel() below)
    xq = nc.dram_tensor("xq", [XB, 128, KC, 256], f32r, kind="ExternalInput")
    xk = nc.dram_tensor("xk", [XB, 128, KC, 256], f32r, kind="ExternalInput")
    xv = nc.dram_tensor("xv", [XB, 128, KC, 256], f32r, kind="ExternalInput")
    wq = nc.dram_tensor("wq", [128, KC, HD2], f32r, kind="ExternalInput")
    wk = nc.dram_tensor("wk", [128, KC, HD2], f32r, kind="ExternalInput")
    wv = nc.dram_tensor("wv", [128, KC, HD2], f32r, kind="ExternalInput")
    wo = nc.dram_tensor("wo", [128, OC, D], f32r, kind="ExternalInput")
    bq = nc.dram_tensor("bq", [128, OC], f32, kind="ExternalInput")
    bk = nc.dram_tensor("bk", [128, OC], f32, kind="ExternalInput")
    bo = nc.dram_tensor("bo", [128, D], f32, kind="ExternalInput")
    mb = nc.dram_tensor("mb", [128, SKC], f32, kind="ExternalInput")   # mask*-1e9
    sel = nc.dram_tensor("sel", [4, NT, OC, 128], f32r, kind="ExternalInput")
    out = nc.dram_tensor("out", [S, D], f32, kind="ExternalOutput")

    with tile.TileContext(nc) as tc:
        _emit(nc, tc, xq, xk, xv, wq, wk, wv, wo, bq, bk, bo, mb, sel, out)
    nc.finalize()
    return nc


def _emit(nc, tc, xq, xk, xv, wq, wk, wv, wo, bq, bk, bo, mb, sel, out):
    from contextlib import ExitStack

    with ExitStack() as ctx:
        consts = ctx.enter_context(tc.tile_pool(name="consts", bufs=1))
        kvres = ctx.enter_context(tc.tile_pool(name="kvres", bufs=1))
        wpool = ctx.enter_context(tc.tile_pool(name="wpool", bufs=2))
        xtp = ctx.enter_context(tc.tile_pool(name="xtp", bufs=2))
        qts = ctx.enter_context(tc.tile_pool(name="qts", bufs=2))
        ptp = ctx.enter_context(tc.tile_pool(name="ptp", bufs=9))
        stg = ctx.enter_context(tc.tile_pool(name="stg", bufs=2))
        stg2 = ctx.enter_context(tc.tile_pool(name="stg2", bufs=2))
        rbp = ctx.enter_context(tc.tile_pool(name="rbp", bufs=2))
        selp = ctx.enter_context(tc.tile_pool(name="selp", bufs=2))
        ctp = ctx.enter_context(tc.tile_pool(name="ctp", bufs=2))
        psA = ctx.enter_context(tc.tile_pool(name="psA", bufs=3, space="PSUM"))
        psC = ctx.enter_context(tc.tile_pool(name="psC", bufs=2, space="PSUM"))
        dram = ctx.enter_context(tc.tile_pool(name="dram", bufs=1, space="DRAM"))

        qtd = dram.tile([HD2, S], f32r)     # Q^T spill
        crd = dram.tile([HD2, S], f32r)     # raw (unnormalized) ctx^T
        dnd = dram.tile([NT, 512], f32r)    # denominator rows

        mb_sb = consts.tile([128, SKC], f32)
        nc.sync.dma_start(mb_sb, mb[:])
        bq_sb = consts.tile([128, OC], f32)
        nc.sync.dma_start(bq_sb, bq[:])
        bk_sb = consts.tile([128, OC], f32)
        nc.sync.dma_start(bk_sb, bk[:])

        kT = kvres.tile([128, OC, S], f32r)          # K^T resident
        va = kvres.tile([128, SKC, 8, DH + 1], f32r)  # V + ones col resident
        nc.vector.memset(va[:, :, :, DH].bitcast(f32), 1.0)

        # ================= phase A =================
        # per-kc DMAs so the first matmul waits on ~256 KB, not 2 MB
        wq_sb = wpool.tile([128, KC, HD2], f32r, tag="w", name="wq_sb")
        for kc in range(KC):
            nc.sync.dma_start(wq_sb[:, kc, :], wq[:, kc, :])
        for blk in range(XB):
            xt = xtp.tile([128, KC, 256], f32r, tag="xT", name="xt")
            if blk == 0:
                for kc in range(KC):
                    nc.sync.dma_start(xt[:, kc, :], xq[blk, :, kc, :])
            else:
                nc.sync.dma_start(xt, xq[blk])
            for dc2 in range(2):
                ps = psA.tile([128, 2, 512], f32, tag="psA", name="ps")
                stq = stg.tile([128, 2, 512], f32r, tag="stg", name="stq")
                for half in range(2):
                    dc = dc2 * 2 + half
                    for kc in range(KC):
                        nc.tensor.matmul(ps[:, half, 0:256],
                                         lhsT=wq_sb[:, kc, dc * 128:(dc + 1) * 128],
                                         rhs=xt[:, kc, :],
                                         start=(kc == 0), stop=(kc == KC - 1))
                    nc.scalar.activation(stq[:, half, 0:256], ps[:, half, 0:256],
                                         Ident, bias=bq_sb[:, dc:dc + 1])
                    nc.sync.dma_start(
                        qtd[dc * 128:(dc + 1) * 128, blk * 256:(blk + 1) * 256],
                        stq[:, half, 0:256])

        wk_sb = wpool.tile([128, KC, HD2], f32r, tag="w", name="wk_sb")
        nc.sync.dma_start(wk_sb, wk[:])
        for blk in range(XB):
            xt = xtp.tile([128, KC, 256], f32r, tag="xT", name="xt")
            nc.sync.dma_start(xt, xk[blk])
            for dc2 in range(2):
                ps = psA.tile([128, 2, 512], f32, tag="psA", name="ps")
                for half in range(2):
                    dc = dc2 * 2 + half
                    for kc in range(KC):
                        nc.tensor.matmul(ps[:, half, 0:256],
                                         lhsT=wk_sb[:, kc, dc * 128:(dc + 1) * 128],
                                         rhs=xt[:, kc, :],
                                         start=(kc == 0), stop=(kc == KC - 1))
                    nc.scalar.activation(
                        kT[:, dc, blk * 256:(blk + 1) * 256],
                        ps[:, half, 0:256], Ident, bias=bk_sb[:, dc:dc + 1])

        wv_sb = wpool.tile([128, KC, HD2], f32r, tag="w", name="wv_sb")
        nc.sync.dma_start(wv_sb, wv[:])
        for blk in range(XB):
            xt = xtp.tile([128, KC, 256], f32r, tag="xT", name="xvt")
            nc.sync.dma_start(xt, xv[blk])
            for sub in range(2):
                sc = blk * 2 + sub
                ps = psC.tile([128, 512], f32, tag="psC", name="psv")
                for kc in range(KC):
                    nc.tensor.matmul(ps,
                                     lhsT=xt[:, kc, sub * 128:(sub + 1) * 128],
                                     rhs=wv_sb[:, kc, :],
                                     start=(kc == 0), stop=(kc == KC - 1))
                with nc.allow_low_precision(reason="V rounded to f32r"):
                    nc.vector.tensor_copy(va[:, sc, :, 0:DH],
                                          ps.rearrange("p (h d) -> p h d", h=8))

        # wo shares the weight ring (viewed [128, OC, 1024]); bo broadcast
        wo_sb_raw = wpool.tile([128, KC, HD2], f32r, tag="w", name="wo_sb")
        nc.sync.dma_start(wo_sb_raw.rearrange("p a b -> p (a b)"),
                          wo[:].rearrange("p a b -> p (a b)"))
        wo_sb = wo_sb_raw.rearrange("p (a c) b -> p a (c b)", c=2)
        bo_sb = consts.tile([128, D], f32)
        nc.sync.dma_start(bo_sb, bo[:])

        # ================= phase B: software-pipelined attention =========
        imap = _iter_map()
        LAG = 6          # ctx trails logits by LAG chunks within a pair-iter
        cur_qt = None
        for i in range(NT // 2):
            ko, sqb = imap[i]
            sq2, sqs = divmod(sqb, 2)
            if sqs == 0:
                cur_qt = qts.tile([128, 1024], f32r, tag="qt", name="qt")
                nc.sync.dma_start(
                    cur_qt,
                    qtd[ko * 128:(ko + 1) * 128, sq2 * 1024:(sq2 + 1) * 1024])
            pts = []
            pscs = [None, None]

            def emit_ctx(sc):
                for h2 in range(2):
                    if sc == 0:
                        pscs[h2] = psC.tile([128, 512], f32, tag="psC",
                                            name="psc")
                    nc.tensor.matmul(pscs[h2][0:DH + 1, :],
                                     lhsT=va[:, sc, ko * 2 + h2, :],
                                     rhs=pts[sc][:, h2, :],
                                     start=(sc == 0), stop=(sc == SKC - 1))

            for sc in range(SKC):
                # both heads' logits for key chunk sc — adjacent matmuls on
                # disjoint PE row groups (base partitions 0 and 64) run
                # concurrently (row tiling)
                psl = psA.tile([128, 2, 512], f32, tag="psA", name="psl")
                for h2 in range(2):
                    b0 = h2 * 64
                    nc.tensor.matmul(
                        psl[:, h2, :],
                        lhsT=kT[b0:b0 + 64, ko, sc * 128:(sc + 1) * 128],
                        rhs=cur_qt[b0:b0 + 64, sqs * 512:(sqs + 1) * 512],
                        start=True, stop=True)
                pt = ptp.tile([128, 2, 512], f32r, tag="pt", name="pt")
                nc.scalar.activation(
                    pt.rearrange("p a b -> p (a b)"),
                    psl.rearrange("p a b -> p (a b)"), Exp,
                    bias=mb_sb[:, sc:sc + 1], scale=0.125)
                pts.append(pt)
                if sc >= LAG:
                    emit_ctx(sc - LAG)
            for sc in range(SKC - LAG, SKC):
                emit_ctx(sc)

            for h2 in range(2):
                h = ko * 2 + h2
                t = 2 * i + h2
                cu = stg2.tile([65, 512], f32r, tag="cu", name="cu")
                with nc.allow_low_precision(reason="raw ctx rounded to f32r"):
                    nc.vector.tensor_copy(cu, pscs[h2][0:DH + 1, :])
                nc.sync.dma_start(
                    crd[h * 64:(h + 1) * 64, sqb * 512:(sqb + 1) * 512],
                    cu[0:DH, :])
                nc.sync.dma_start(dnd[t:t + 1, :], cu[DH:DH + 1, :])

        # ================= phase C: normalize + output projection =========
        den_sb = consts.tile([NT, 512], f32r)
        nc.sync.dma_start(den_sb, dnd[:])
        recf = consts.tile([NT, 512], f32)
        nc.vector.reciprocal(recf, den_sb)
        rec = consts.tile([NT, 512], f32r)
        with nc.allow_low_precision(reason="softmax recip rounded to f32r"):
            nc.vector.tensor_copy(rec, recf)

        for sqb in range(4):
            sel_t = selp.tile([NT, OC, 128], f32r, tag="sel", name="sel_t")
            nc.sync.dma_start(sel_t, sel[sqb])
            rbt = rbp.tile([128, OC, 512], f32r, tag="rb", name="rbt")
            for ko in range(OC):
                pb = psC.tile([128, 512], f32, tag="psC", name="pb")
                nc.tensor.matmul(pb, lhsT=sel_t[:, ko, :], rhs=rec[:],
                                 start=True, stop=True)
                with nc.allow_low_precision(reason="recip bcast in f32r"):
                    nc.vector.tensor_copy(rbt[:, ko, :], pb)
            for st4 in range(4):
                st8 = sqb * 4 + st4
                cT = ctp.tile([128, OC, 128], f32r, tag="cT", name="cT")
                nc.sync.dma_start(cT, crd[:, st8 * 128:(st8 + 1) * 128]
                                  .rearrange("(ko p) q -> p ko q", p=128))
                with nc.allow_low_precision(reason="normalized ctx in f32r"):
                    nc.vector.tensor_mul(out=cT, in0=cT,
                                         in1=rbt[:, :, st4 * 128:(st4 + 1) * 128])
                ps = psA.tile([128, 2, 512], f32, tag="psA", name="pso")
                for half in range(2):
                    for ko in range(OC):
                        nc.tensor.matmul(
                            ps[:, half, :], lhsT=cT[:, ko, :],
                            rhs=wo_sb[:, ko, half * 512:(half + 1) * 512],
                            start=(ko == 0), stop=(ko == OC - 1))
                st_t = stg.tile([128, 2, 512], f32r, tag="stg", name="ost")
                with nc.allow_low_precision(reason="f32r storage is fp32 bits"):
                    nc.vector.tensor_add(
                        out=st_t.rearrange("p a b -> p (a b)"),
                        in0=ps.rearrange("p a b -> p (a b)"), in1=bo_sb)
                nc.sync.dma_start(out[st8 * 128:(st8 + 1) * 128, :],
                                  st_t.rearrange("p a b -> p (a b)").bitcast(f32))


_NC_CACHE = None


def _selector():
    # sel[sqb, t, ko, p] = 1 iff phase-B denominator row t holds
    # (head ko*2 + p//64, query block sqb); t = 2*pair_iter + h2
    imap = _iter_map()
    s = np.zeros((4, NT, OC, 128), np.float32)
    for i, (ko, sqb) in enumerate(imap):
        for h2 in range(2):
            s[sqb, 2 * i + h2, ko, h2 * 64:(h2 + 1) * 64] = 1.0
    return s


def kernel(query, key, value, mask, wq, bq, wk, bk, wv, bv, wo, bo):
    global _NC_CACHE
    if _NC_CACHE is None:
        _NC_CACHE = _build()
    nc = _NC_CACHE

    query = np.asarray(query, dtype=np.float32)
    key = np.asarray(key, dtype=np.float32)
    value = np.asarray(value, dtype=np.float32)
    mask = np.asarray(mask, dtype=np.float32)
    wq_np = np.asarray(wq, np.float32)
    wk_np = np.asarray(wk, np.float32)
    wv_np = np.asarray(wv, np.float32)
    wo_np = np.asarray(wo, np.float32)
    bq_np = np.asarray(bq, np.float32)
    bk_np = np.asarray(bk, np.float32)
    bv_np = np.asarray(bv, np.float64)
    bo_np = np.asarray(bo, np.float64)

    def chunk_x(xT):
        # [1024, 2048] -> [XB, 128, KC, 256] with d = kc*128 + p
        return np.ascontiguousarray(
            xT.reshape(KC, 128, XB, 256).transpose(2, 1, 0, 3))

    def chunk_w(w_half):
        # [1024, 512] -> [128, KC, 512]
        return np.ascontiguousarray(
            w_half.reshape(KC, 128, HD2).transpose(1, 0, 2))

    xq_b, xk_b, xv_b = [], [], []
    for b in range(B):
        xq_b.append(chunk_x(np.ascontiguousarray(query[b].T)))
        xk_b.append(chunk_x(np.ascontiguousarray(key[b].T)))
        xv_b.append(chunk_x(np.ascontiguousarray(value[b].T)))

    sel_host = _selector()
    in_maps = []
    for core in range(N_CORES):
        b, hg = divmod(core, 2)
        sl = slice(hg * HD2, (hg + 1) * HD2)
        bias_out = bv_np[sl] @ wo_np[sl].astype(np.float64)
        if hg == 0:
            bias_out = bias_out + bo_np
        mbc = np.ascontiguousarray(
            (mask[b, 0, 0] * np.float32(-1e9)).reshape(SKC, 128).T)
        in_maps.append({
            "xq": xq_b[b], "xk": xk_b[b], "xv": xv_b[b],
            "wq": chunk_w(wq_np[:, sl]),
            "wk": chunk_w(wk_np[:, sl]),
            "wv": chunk_w(wv_np[:, sl]),
            "wo": np.ascontiguousarray(
                wo_np[sl].reshape(OC, 128, D).transpose(1, 0, 2)),
            "bq": np.ascontiguousarray(bq_np[sl].reshape(OC, 128).T),
            "bk": np.ascontiguousarray(bk_np[sl].reshape(OC, 128).T),
            "bo": np.ascontiguousarray(
                np.broadcast_to(bias_out.astype(np.float32), (128, D))),
            "mb": mbc, "sel": sel_host,
        })

    res = run_bass_kernel_spmd(nc, in_maps, core_ids=list(range(N_CORES)))
    full = np.empty((B, S, D), np.float32)
    for b in range(B):
        full[b] = res.results[2 * b]["out"]
        full[b] += res.results[2 * b + 1]["out"]
    return full
